# revision 2
# baseline (speedup 1.0000x reference)
"""HMM forward-algorithm kernel for Trainium2 (8 NeuronCores), fp8 edition.

Strategy
--------
The unnormalized HMM forward recurrence  alpha_{t+1} = (alpha_t @ A) * em_{t+1}
is linear in alpha, and A = softmax(randn) mixes fast (|lambda_2| ~ 1/sqrt(S)),
so the scan over T=2048 steps is split into C=128 time-chunks, each warmed up
for W=1 step from a uniform state: after warmup the state has converged to the
true forward state far below the error budget.  All 128 chunks x 32 batch
elements form independent recurrences, distributed over 8 cores as 512 columns
per core.  Each core runs ITERS=17 steps of  alphaT <- (A'^T @ alphaT) .* em
on a [S=512, N=512] state.

fp8 arithmetic: A' = fp8e4m3(32*A) packed as DoubleRow weight blocks (K=256
per matmul -> 8 scan matmuls/iter instead of 16 bf16 ones), alpha carried in
fp8e4m3 (validated vs float64 on the fixed-seed data: |alpha| in [4e-3, 3.8],
far from the e4m3 +-240 range; end-to-end maxerr ~4 on an output of ~7100,
rel ~6e-4).  Emissions are produced on the PE as Bem^T-replica @ one-hot fp8
matmuls (a gather in disguise), copied PSUM->SBUF bf16 on the scalar engine,
and multiplied into the scan PSUM output on the DVE, writing fp8 alpha.

No per-step normalization: the 32x scale in A' keeps column sums ~O(1), and
the host subtracts n_steps*log(32) exactly.  Instead of on-device colsums,
the full fp8 alpha tiles are DMA'd to DRAM at 3 snapshot iterations and the
host computes colsums + telescoped  sum_t log z_t = log(cs_end) - log(cs_start)
per chunk in float64.
"""

import os
import sys
from contextlib import ExitStack

import numpy as np

for _p in ("/root/.axon_site", "/root/.axon_site/_ro/trn_rl_repo", "/opt/trn_rl_repo"):
    if os.path.isdir(_p) and _p not in sys.path:
        sys.path.append(_p)

import ml_dtypes

BF16 = ml_dtypes.bfloat16
E4 = ml_dtypes.float8_e4m3

# Problem shape (hardcoded per contract).
B, T, S, E = 32, 2048, 512, 32
NCORES = 8
NCH = 16              # time-chunks per core
C = NCORES * NCH      # 128 global chunks
W = 1                 # warmup steps per chunk
L = 16                # nominal own-steps per chunk
ITERS = W + L         # 17 device iterations
N = NCH * B           # 512 columns per core
SNAPS = (W - 1, ITERS - 2, ITERS - 1)
A_SCALE = 32.0        # folded into A on host; host subtracts n*log(32)
INIT_SCALE = np.float32(2.0 ** -3) * S  # column mean alpha entry ~2^-3
_CACHE = {}


def _plan():
    """Global chunk partition of own-step ranges covering t in [1, T-1]."""
    need = (T - 1) - (W + L)          # steps owned by chunks 1..C-1
    a_full = need - (L - 1) * (C - 1)  # chunks owning L steps
    assert 0 <= a_full <= C - 1
    own_len = [W + L] + [L] * a_full + [L - 1] * ((C - 1) - a_full)
    starts = [1]
    for c in range(1, C):
        starts.append(starts[c - 1] + own_len[c - 1])
    assert starts[-1] + own_len[-1] - 1 == T - 1
    tbase = [1] + [starts[c] - W for c in range(1, C)]
    return own_len, tbase


def _build():
    """Build + compile the per-core Bass program (identical across cores)."""
    from concourse import bacc, mybir
    import concourse.tile as tile

    nc = bacc.Bacc("TRN2", target_bir_lowering=False, debug=False)
    bf = mybir.dt.bfloat16
    f32 = mybir.dt.float32
    f8 = mybir.dt.float8e4
    DR = mybir.MatmulPerfMode.DoubleRow

    # A' = fp8(32*A) packed as 8 DoubleRow blocks b=(K*4+m): [128, 2, 128]
    # with a8[p, b, i, m'] = A'[(2K+i)*128+p, m*128+m'].
    a_d = nc.dram_tensor("a8", (128, 8, 2, 128), f8, kind="ExternalInput").ap()
    # Bem^T tiled 4x vertically (K=128 one-hot emission matmuls; one-hot rows
    # offset by 32*(iter%4) to select a replica).
    bemt_d = nc.dram_tensor("bemt8", (128, S), f8, kind="ExternalInput").ap()
    x_d = nc.dram_tensor("x8", (128, ITERS, N), f8, kind="ExternalInput").ap()
    # init8[p, kt, col] = alpha_init[kt*128+p, col]
    init_d = nc.dram_tensor("init8", (128, 4, N), f8, kind="ExternalInput").ap()
    out_d = nc.dram_tensor("snaps8", (len(SNAPS), 128, 4, N), f8,
                           kind="ExternalOutput").ap()

    with tile.TileContext(nc) as tc, ExitStack() as ctx:
        consts = ctx.enter_context(tc.tile_pool(name="consts", bufs=1))
        alphap = ctx.enter_context(tc.tile_pool(name="alpha", bufs=2))
        emp = ctx.enter_context(tc.tile_pool(name="em", bufs=2))
        pscan = ctx.enter_context(tc.tile_pool(name="pscan", bufs=1, space="PSUM"))
        pem = ctx.enter_context(tc.tile_pool(name="pem", bufs=1, space="PSUM"))

        # PSUM: 4 banks for the scan output, 4 for the emission products.
        ps = pscan.tile([128, 4 * N], f32, tag="ps", name="ps")
        ep = pem.tile([128, 4 * N], f32, tag="ep", name="ep")

        # PE warmup: HAM keeps the PE at 1.2 GHz until ~3.4us of sustained
        # ARRAY activity.  Dense full-array dummy matmuls on a zeroed tile
        # warm it while input DMAs are in flight.  They write the em PSUM
        # banks (WAR deps against the first real em matmuls order them).
        dummy_w = consts.tile([128, S], bf, tag="dummy", name="dummy_w")
        nc.vector.memset(dummy_w, 0.0)
        dummy_n = [0]

        def emit_dummy(count):
            for _ in range(count):
                r = dummy_n[0] % 4
                dummy_n[0] += 1
                nc.tensor.matmul(
                    ps[:, r * N:(r + 1) * N], dummy_w[:, 0:128], dummy_w[:],
                    start=True, stop=True,
                )

        emit_dummy(10)

        # Input loads: em dependencies (Bem, first X slices) first, then
        # init/A so the first scan iteration can start, then the X tail.
        bemt_sb = consts.tile([128, S], f8, tag="bemt", name="bemt")
        nc.default_dma_engine.dma_start(out=bemt_sb, in_=bemt_d[:, :])
        x_sb = consts.tile([128, ITERS, N], f8, tag="xoh", name="xoh")
        nc.default_dma_engine.dma_start(out=x_sb[:, 0:2, :], in_=x_d[:, 0:2, :])
        init_sb = consts.tile([128, 4, N], f8, tag="init", name="init_sb")
        nc.default_dma_engine.dma_start(out=init_sb, in_=init_d[:, :, :])
        a_sb = consts.tile([128, 8, 2, 128], f8, tag="a", name="a_sb")
        nc.default_dma_engine.dma_start(out=a_sb, in_=a_d[:, :, :, :])
        nc.default_dma_engine.dma_start(
            out=x_sb[:, 2:ITERS, :], in_=x_d[:, 2:ITERS, :]
        )

        # alpha views: kpair K covers state rows (2K+i)*128+p, i in {0,1}
        alpha = [init_sb[:, 0:2, :], init_sb[:, 2:4, :]]

        snap_row = 0
        for i in range(ITERS):
            # Emission products first: independent of alpha, they fill the
            # PE while the DVE finishes the previous iteration's multiplies.
            for m in range(4):
                nc.tensor.matmul(
                    ep[:, m * N:(m + 1) * N],
                    bemt_sb[:, m * 128:(m + 1) * 128],
                    x_sb[:, i, :],
                    start=True, stop=True,
                )
            em_t = emp.tile([128, 4 * N], bf, tag="em", name=f"em_{i}")
            for h in range(2):
                nc.scalar.copy(
                    em_t[:, h * 2 * N:(h + 1) * 2 * N],
                    ep[:, h * 2 * N:(h + 1) * 2 * N],
                )
            # Scan: m-outer, kpair-inner, DoubleRow fp8 (K=256 per matmul).
            for m in range(4):
                for K in range(2):
                    nc.tensor.matmul(
                        ps[:, m * N:(m + 1) * N],
                        a_sb[:, K * 4 + m, :, :],
                        alpha[K],
                        start=(K == 0), stop=(K == 1),
                        perf_mode=DR,
                    )
            new_alpha = []
            for K in range(2):
                t = alphap.tile([128, 2, N], f8, tag=f"al{K}", name=f"al_{i}_{K}")
                for j in range(2):
                    m = 2 * K + j
                    nc.vector.tensor_mul(
                        t[:, j:j + 1, :],
                        ps[:, m * N:(m + 1) * N],
                        em_t[:, m * N:(m + 1) * N],
                    )
                new_alpha.append(t)
            alpha = new_alpha
            if i in SNAPS:
                for K in range(2):
                    nc.default_dma_engine.dma_start(
                        out=out_d[snap_row, :, K * 2:(K + 1) * 2, :],
                        in_=alpha[K],
                    )
                snap_row += 1

    nc.compile()
    return nc


def _get_nc():
    if "nc" not in _CACHE:
        _CACHE["nc"] = _build()
    return _CACHE["nc"]


def _pack(inputs, A, Bem, pi):
    """Host-side input prep: shard chunks over cores, build fp8 inputs."""
    own_len, tbase = _plan()
    obs = np.ascontiguousarray(np.argmax(inputs, axis=-1))  # [B, T]

    # A' = fp8(32*A) -> DoubleRow blocks [128, 8, 2, 128]
    Aq = (np.float32(A_SCALE) * A).astype(E4)
    A4 = Aq.reshape(4, 128, 4, 128)            # [kt, p, m, m']
    a8 = np.empty((128, 8, 2, 128), E4)
    for Kp in range(2):
        for m in range(4):
            for i2 in range(2):
                a8[:, Kp * 4 + m, i2, :] = A4[2 * Kp + i2, :, m, :]
    a8 = np.ascontiguousarray(a8)

    bemt8 = np.ascontiguousarray(np.tile(Bem.astype(E4).T, (4, 1)))  # [128, S]

    # chunk-0 init column (true normalized alpha_0), other chunks uniform.
    em0 = Bem[np.arange(S)[:, None], obs[None, :, 0]]       # [S, B]
    alpha0 = pi[:, None] * em0
    z0 = alpha0.sum(axis=0, dtype=np.float64)               # [B]
    alpha0n = alpha0 / z0.astype(np.float32)

    tb = np.asarray(tbase)
    in_maps = []
    s0_chunk0 = None
    for core in range(NCORES):
        tbs = tb[core * NCH:(core + 1) * NCH]               # [NCH]
        t_idx = np.clip(tbs[None, :] + np.arange(ITERS)[:, None], 1, T - 1)
        sym = obs[:, t_idx]                                 # [B, ITERS, NCH]
        sym = np.moveaxis(sym, 0, 2)                        # [ITERS, NCH, B]
        sym = sym.reshape(ITERS, N)
        sym = sym + (np.arange(ITERS) % 4)[:, None] * E     # replica row offset
        x8 = (sym[None, :, :] == np.arange(128)[:, None, None]).astype(E4)
        x8 = np.ascontiguousarray(x8)                       # [128, ITERS, N]

        init = np.full((S, N), INIT_SCALE / np.float32(S), np.float32)
        if core == 0:
            init[:, 0:B] = alpha0n * INIT_SCALE
        init8 = init.astype(E4)
        if core == 0:
            s0_chunk0 = np.log(init8[:, 0:B].astype(np.float64).sum(axis=0))
        init8 = np.ascontiguousarray(
            init8.reshape(4, 128, N).transpose(1, 0, 2)     # [p, kt, col]
        )
        in_maps.append({
            "a8": a8,
            "bemt8": bemt8,
            "x8": x8,
            "init8": init8,
        })

    host = {"own_len": own_len, "z0": z0, "s0_chunk0": s0_chunk0}
    return in_maps, host


def _assemble(results, host):
    """Combine per-core alpha snapshots into loglik [B] (float64 host math)."""
    own_len = host["own_len"]
    logc = np.log(np.float64(A_SCALE))
    loglik = np.log(host["z0"]).copy()                      # [B]
    for core in range(NCORES):
        snaps = results[core]["snaps8"]                     # [3, 128, 4, N] fp8
        al = snaps.astype(np.float64).transpose(0, 2, 1, 3).reshape(3, S, N)
        lz = np.log(al.sum(axis=1))                         # [3, N]
        for cl in range(NCH):
            c = core * NCH + cl
            cols = slice(cl * B, (cl + 1) * B)
            if c == 0:
                nst = own_len[0]
                loglik += lz[2, cols] - host["s0_chunk0"] - nst * logc
            else:
                row = 2 if own_len[c] == L else 1
                nst = (ITERS - 1 if row == 2 else ITERS - 2) - (W - 1)
                loglik += lz[row, cols] - lz[0, cols] - nst * logc
    return loglik.astype(np.float32)


def run(inputs, A, Bem, pi, trace=False):
    from concourse import bass_utils

    nc = _get_nc()
    in_maps, host = _pack(
        np.asarray(inputs, np.float32), np.asarray(A, np.float32),
        np.asarray(Bem, np.float32), np.asarray(pi, np.float32),
    )
    res = bass_utils.run_bass_kernel_spmd(
        nc, in_maps, core_ids=list(range(NCORES)), trace=trace
    )
    loglik = _assemble(res.results, host)
    return loglik, res


def kernel(inputs, A, Bem, pi):
    loglik, _ = run(inputs, A, Bem, pi, trace=False)
    return loglik


# revision 3
# speedup vs baseline: 1.4201x; 1.4201x over previous
"""HMM forward-algorithm kernel for Trainium2 (8 NeuronCores), fp8 edition.

Strategy
--------
The unnormalized HMM forward recurrence  alpha_{t+1} = (alpha_t @ A) * em_{t+1}
is linear in alpha, and A = softmax(randn) mixes fast (|lambda_2| ~ 1/sqrt(S)),
so the scan over T=2048 steps is split into C=128 time-chunks, each warmed up
for W=1 step from a uniform state: after warmup the state has converged to the
true forward state far below the error budget.  All 128 chunks x 32 batch
elements form independent recurrences, distributed over 8 cores as 512 columns
per core.  Each core runs ITERS=17 steps of  alphaT <- (A'^T @ alphaT) .* em
on a [S=512, N=512] state.

fp8 arithmetic: A' = fp8e4m3(32*A) packed as DoubleRow weight blocks (K=256
per matmul -> 8 scan matmuls/iter instead of 16 bf16 ones), alpha carried in
fp8e4m3 (validated vs float64 on the fixed-seed data: |alpha| in [4e-3, 3.8],
far from the e4m3 +-240 range; end-to-end maxerr ~4 on an output of ~7100,
rel ~6e-4).  Emissions are produced on the PE as Bem^T-replica @ one-hot fp8
matmuls (a gather in disguise), copied PSUM->SBUF bf16 on the scalar engine,
and multiplied into the scan PSUM output on the DVE, writing fp8 alpha.

No per-step normalization: the 32x scale in A' keeps column sums ~O(1), and
the host subtracts n_steps*log(32) exactly.  Instead of on-device colsums,
the full fp8 alpha tiles are DMA'd to DRAM at 3 snapshot iterations and the
host computes colsums + telescoped  sum_t log z_t = log(cs_end) - log(cs_start)
per chunk in float64.
"""

import os
import sys
from contextlib import ExitStack

import numpy as np

for _p in ("/root/.axon_site", "/root/.axon_site/_ro/trn_rl_repo", "/opt/trn_rl_repo"):
    if os.path.isdir(_p) and _p not in sys.path:
        sys.path.append(_p)

import ml_dtypes

BF16 = ml_dtypes.bfloat16
E4 = ml_dtypes.float8_e4m3

# Problem shape (hardcoded per contract).
B, T, S, E = 32, 2048, 512, 32
NCORES = 8
NCH = 16              # time-chunks per core
C = NCORES * NCH      # 128 global chunks
W = 1                 # warmup steps per chunk
L = 16                # nominal own-steps per chunk
ITERS = W + L         # 17 device iterations
N = NCH * B           # 512 columns per core
SNAPS = (W - 1, ITERS - 2, ITERS - 1)
A_SCALE = 32.0        # folded into A on host; host subtracts n*log(32)
INIT_SCALE = np.float32(2.0 ** -3) * S  # column mean alpha entry ~2^-3
_CACHE = {}


def _plan():
    """Global chunk partition of own-step ranges covering t in [1, T-1]."""
    need = (T - 1) - (W + L)          # steps owned by chunks 1..C-1
    a_full = need - (L - 1) * (C - 1)  # chunks owning L steps
    assert 0 <= a_full <= C - 1
    own_len = [W + L] + [L] * a_full + [L - 1] * ((C - 1) - a_full)
    starts = [1]
    for c in range(1, C):
        starts.append(starts[c - 1] + own_len[c - 1])
    assert starts[-1] + own_len[-1] - 1 == T - 1
    tbase = [1] + [starts[c] - W for c in range(1, C)]
    return own_len, tbase


def _build():
    """Build + compile the per-core Bass program (identical across cores)."""
    from concourse import bacc, mybir
    import concourse.tile as tile

    nc = bacc.Bacc("TRN2", target_bir_lowering=False, debug=False)
    bf = mybir.dt.bfloat16
    f32 = mybir.dt.float32
    f8 = mybir.dt.float8e4
    DR = mybir.MatmulPerfMode.DoubleRow

    # A' = fp8(32*A) packed as 8 DoubleRow blocks b=(K*4+m): [128, 2, 128]
    # with a8[p, b, i, m'] = A'[(2K+i)*128+p, m*128+m'].
    a_d = nc.dram_tensor("a8", (128, 8, 2, 128), f8, kind="ExternalInput").ap()
    # Bem^T tiled 4x vertically (K=128 one-hot emission matmuls; one-hot rows
    # offset by 32*(iter%4) to select a replica).
    bemt_d = nc.dram_tensor("bemt8", (128, S), f8, kind="ExternalInput").ap()
    x_d = nc.dram_tensor("x8", (128, ITERS, N), f8, kind="ExternalInput").ap()
    # init8[p, kt, col] = alpha_init[kt*128+p, col]
    init_d = nc.dram_tensor("init8", (128, 4, N), f8, kind="ExternalInput").ap()
    out_d = nc.dram_tensor("snaps8", (len(SNAPS), 128, 4, N), f8,
                           kind="ExternalOutput").ap()

    with tile.TileContext(nc) as tc, ExitStack() as ctx:
        consts = ctx.enter_context(tc.tile_pool(name="consts", bufs=1))
        alphap = ctx.enter_context(tc.tile_pool(name="alpha", bufs=2))
        emp = ctx.enter_context(tc.tile_pool(name="em", bufs=4))
        pscan = ctx.enter_context(tc.tile_pool(name="pscan", bufs=1, space="PSUM"))
        pem = ctx.enter_context(tc.tile_pool(name="pem", bufs=2, space="PSUM"))

        # Scan PSUM: one tile per kpair half (m=2K, 2K+1) so the DVE multiply
        # for half K depends only on its own 4 matmuls (Tile tracks deps at
        # tile granularity - a single fused tile would serialize everything).
        ps_h = [
            pscan.tile([128, 2 * N], f32, tag=f"ps{K}", name=f"ps{K}")
            for K in range(2)
        ]

        # PE warmup: HAM keeps the PE at 1.2 GHz until ~3.4us of sustained
        # ARRAY activity.  Dense full-array dummy matmuls on a zeroed tile
        # warm it while input DMAs are in flight (WAR deps against the first
        # real scan matmuls order them).
        dummy_w = consts.tile([128, S], bf, tag="dummy", name="dummy_w")
        nc.vector.memset(dummy_w, 0.0)
        dummy_n = [0]

        def emit_dummy(count):
            for _ in range(count):
                r = dummy_n[0] % 2
                dummy_n[0] += 1
                nc.tensor.matmul(
                    ps_h[r][:, 0:N], dummy_w[:, 0:128], dummy_w[:],
                    start=True, stop=True,
                )

        emit_dummy(6)

        # Input loads: em dependencies (Bem, first X slices) first, then
        # init/A so the first scan iteration can start, then the X tail.
        bemt_sb = consts.tile([128, S], f8, tag="bemt", name="bemt")
        nc.default_dma_engine.dma_start(out=bemt_sb, in_=bemt_d[:, :])
        x_sb = consts.tile([128, ITERS, N], f8, tag="xoh", name="xoh")
        nc.default_dma_engine.dma_start(out=x_sb[:, 0:4, :], in_=x_d[:, 0:4, :])
        init_sb = consts.tile([128, 4, N], f8, tag="init", name="init_sb")
        nc.default_dma_engine.dma_start(out=init_sb, in_=init_d[:, :, :])
        a_sb = consts.tile([128, 8, 2, 128], f8, tag="a", name="a_sb")
        nc.default_dma_engine.dma_start(out=a_sb, in_=a_d[:, :, :, :])
        nc.default_dma_engine.dma_start(
            out=x_sb[:, 4:ITERS, :], in_=x_d[:, 4:ITERS, :]
        )

        def emit_em(i, prologue=False):
            """Emission products for iter i: 4 one-hot matmuls + 2 ACT copies.

            pem bufs=2 paces the matmuls against the copies; em_t bufs=4
            holds iters i..i+3 (emitted 3 iterations ahead in the loop)."""
            halves = []
            for h in range(2):
                ep = pem.tile([128, 2 * N], f32, tag="pem", name=f"ep_{i}_{h}")
                for j in range(2):
                    m = 2 * h + j
                    nc.tensor.matmul(
                        ep[:, j * N:(j + 1) * N],
                        bemt_sb[:, m * 128:(m + 1) * 128],
                        x_sb[:, i, :],
                        start=True, stop=True,
                    )
                et = emp.tile([128, 2 * N], bf, tag=f"emh{h}", name=f"em_{i}_{h}")
                nc.scalar.copy(et[:], ep[:])
                if prologue:
                    emit_dummy(3)
                halves.append(et)
            return halves

        em_tiles = {j: emit_em(j, prologue=True) for j in range(3)}

        # alpha views: kpair K covers state rows (2K+i)*128+p, i in {0,1}
        alpha = [init_sb[:, 0:2, :], init_sb[:, 2:4, :]]

        snap_row = 0
        for i in range(ITERS):
            # Emission products for iter i+3 first: independent of alpha,
            # they fill the PE while the DVE finishes iter i-1's multiplies.
            if i + 3 < ITERS:
                em_tiles[i + 3] = emit_em(i + 3)
            # Scan: DoubleRow fp8 (K=256 per matmul).  The first two matmuls
            # consume only alpha[0] so iter i-1's alpha[1] multiply gets an
            # extra ~2 matmul slots of slack.
            for m, K in ((0, 0), (1, 0), (0, 1), (1, 1),
                         (2, 0), (3, 0), (2, 1), (3, 1)):
                nc.tensor.matmul(
                    ps_h[m // 2][:, (m % 2) * N:(m % 2 + 1) * N],
                    a_sb[:, K * 4 + m, :, :],
                    alpha[K],
                    start=(K == 0), stop=(K == 1),
                    perf_mode=DR,
                )
            new_alpha = []
            for K in range(2):
                t = alphap.tile([128, 2, N], f8, tag=f"al{K}", name=f"al_{i}_{K}")
                nc.vector.tensor_mul(t[:], ps_h[K][:], em_tiles[i][K][:])
                new_alpha.append(t)
            del em_tiles[i]
            alpha = new_alpha
            if i in SNAPS:
                for K in range(2):
                    nc.default_dma_engine.dma_start(
                        out=out_d[snap_row, :, K * 2:(K + 1) * 2, :],
                        in_=alpha[K],
                    )
                snap_row += 1

    nc.compile()
    return nc


def _get_nc():
    if "nc" not in _CACHE:
        _CACHE["nc"] = _build()
    return _CACHE["nc"]


def _pack(inputs, A, Bem, pi):
    """Host-side input prep: shard chunks over cores, build fp8 inputs."""
    own_len, tbase = _plan()
    obs = np.ascontiguousarray(np.argmax(inputs, axis=-1))  # [B, T]

    # A' = fp8(32*A) -> DoubleRow blocks [128, 8, 2, 128]
    Aq = (np.float32(A_SCALE) * A).astype(E4)
    A4 = Aq.reshape(4, 128, 4, 128)            # [kt, p, m, m']
    a8 = np.empty((128, 8, 2, 128), E4)
    for Kp in range(2):
        for m in range(4):
            for i2 in range(2):
                a8[:, Kp * 4 + m, i2, :] = A4[2 * Kp + i2, :, m, :]
    a8 = np.ascontiguousarray(a8)

    bemt8 = np.ascontiguousarray(np.tile(Bem.astype(E4).T, (4, 1)))  # [128, S]

    # chunk-0 init column (true normalized alpha_0), other chunks uniform.
    em0 = Bem[np.arange(S)[:, None], obs[None, :, 0]]       # [S, B]
    alpha0 = pi[:, None] * em0
    z0 = alpha0.sum(axis=0, dtype=np.float64)               # [B]
    alpha0n = alpha0 / z0.astype(np.float32)

    tb = np.asarray(tbase)
    in_maps = []
    s0_chunk0 = None
    for core in range(NCORES):
        tbs = tb[core * NCH:(core + 1) * NCH]               # [NCH]
        t_idx = np.clip(tbs[None, :] + np.arange(ITERS)[:, None], 1, T - 1)
        sym = obs[:, t_idx]                                 # [B, ITERS, NCH]
        sym = np.moveaxis(sym, 0, 2)                        # [ITERS, NCH, B]
        sym = sym.reshape(ITERS, N)
        sym = sym + (np.arange(ITERS) % 4)[:, None] * E     # replica row offset
        x8 = (sym[None, :, :] == np.arange(128)[:, None, None]).astype(E4)
        x8 = np.ascontiguousarray(x8)                       # [128, ITERS, N]

        init = np.full((S, N), INIT_SCALE / np.float32(S), np.float32)
        if core == 0:
            init[:, 0:B] = alpha0n * INIT_SCALE
        init8 = init.astype(E4)
        if core == 0:
            s0_chunk0 = np.log(init8[:, 0:B].astype(np.float64).sum(axis=0))
        init8 = np.ascontiguousarray(
            init8.reshape(4, 128, N).transpose(1, 0, 2)     # [p, kt, col]
        )
        in_maps.append({
            "a8": a8,
            "bemt8": bemt8,
            "x8": x8,
            "init8": init8,
        })

    host = {"own_len": own_len, "z0": z0, "s0_chunk0": s0_chunk0}
    return in_maps, host


def _assemble(results, host):
    """Combine per-core alpha snapshots into loglik [B] (float64 host math)."""
    own_len = host["own_len"]
    logc = np.log(np.float64(A_SCALE))
    loglik = np.log(host["z0"]).copy()                      # [B]
    for core in range(NCORES):
        snaps = results[core]["snaps8"]                     # [3, 128, 4, N] fp8
        al = snaps.astype(np.float64).transpose(0, 2, 1, 3).reshape(3, S, N)
        lz = np.log(al.sum(axis=1))                         # [3, N]
        for cl in range(NCH):
            c = core * NCH + cl
            cols = slice(cl * B, (cl + 1) * B)
            if c == 0:
                nst = own_len[0]
                loglik += lz[2, cols] - host["s0_chunk0"] - nst * logc
            else:
                row = 2 if own_len[c] == L else 1
                nst = (ITERS - 1 if row == 2 else ITERS - 2) - (W - 1)
                loglik += lz[row, cols] - lz[0, cols] - nst * logc
    return loglik.astype(np.float32)


def run(inputs, A, Bem, pi, trace=False):
    from concourse import bass_utils

    nc = _get_nc()
    in_maps, host = _pack(
        np.asarray(inputs, np.float32), np.asarray(A, np.float32),
        np.asarray(Bem, np.float32), np.asarray(pi, np.float32),
    )
    res = bass_utils.run_bass_kernel_spmd(
        nc, in_maps, core_ids=list(range(NCORES)), trace=trace
    )
    loglik = _assemble(res.results, host)
    return loglik, res


def kernel(inputs, A, Bem, pi):
    loglik, _ = run(inputs, A, Bem, pi, trace=False)
    return loglik


# revision 6
# speedup vs baseline: 1.4398x; 1.0138x over previous
"""HMM forward-algorithm kernel for Trainium2 (8 NeuronCores), fp8 edition.

Strategy
--------
The unnormalized HMM forward recurrence  alpha_{t+1} = (alpha_t @ A) * em_{t+1}
is linear in alpha, and A = softmax(randn) mixes fast (|lambda_2| ~ 1/sqrt(S)),
so the scan over T=2048 steps is split into C=128 time-chunks, each warmed up
for W=1 step from a uniform state: after warmup the state has converged to the
true forward state far below the error budget.  All 128 chunks x 32 batch
elements form independent recurrences, distributed over 8 cores as 512 columns
per core.  Each core runs ITERS=17 steps of  alphaT <- (A'^T @ alphaT) .* em
on a [S=512, N=512] state.

fp8 arithmetic: A' = fp8e4m3(32*A) packed as DoubleRow weight blocks (K=256
per matmul -> 8 scan matmuls/iter instead of 16 bf16 ones), alpha carried in
fp8e4m3 (validated vs float64 on the fixed-seed data: |alpha| in [4e-3, 3.8],
far from the e4m3 +-240 range; end-to-end maxerr ~4 on an output of ~7100,
rel ~6e-4).  Emissions are produced on the PE as Bem^T-replica @ one-hot fp8
matmuls (a gather in disguise), copied PSUM->SBUF bf16 on the scalar engine,
and multiplied into the scan PSUM output on the DVE, writing fp8 alpha.

No per-step normalization: the 32x scale in A' keeps column sums ~O(1), and
the host subtracts n_steps*log(32) exactly.  Instead of on-device colsums,
the full fp8 alpha tiles are DMA'd to DRAM at 3 snapshot iterations and the
host computes colsums + telescoped  sum_t log z_t = log(cs_end) - log(cs_start)
per chunk in float64.
"""

import os
import sys
from contextlib import ExitStack

import numpy as np

for _p in ("/root/.axon_site", "/root/.axon_site/_ro/trn_rl_repo", "/opt/trn_rl_repo"):
    if os.path.isdir(_p) and _p not in sys.path:
        sys.path.append(_p)

import ml_dtypes

BF16 = ml_dtypes.bfloat16
E4 = ml_dtypes.float8_e4m3

# Problem shape (hardcoded per contract).
B, T, S, E = 32, 2048, 512, 32
NCORES = 8
NCH = 16              # time-chunks per core
C = NCORES * NCH      # 128 global chunks
W = 1                 # warmup steps per chunk
L = 16                # nominal own-steps per chunk
ITERS = W + L         # 17 device iterations
N = NCH * B           # 512 columns per core
SNAPS = (W - 1, ITERS - 2, ITERS - 1)
A_SCALE = 32.0        # folded into A on host; host subtracts n*log(32)
INIT_SCALE = np.float32(2.0 ** -3) * S  # column mean alpha entry ~2^-3
_CACHE = {}


def _plan():
    """Global chunk partition of own-step ranges covering t in [1, T-1]."""
    need = (T - 1) - (W + L)          # steps owned by chunks 1..C-1
    a_full = need - (L - 1) * (C - 1)  # chunks owning L steps
    assert 0 <= a_full <= C - 1
    own_len = [W + L] + [L] * a_full + [L - 1] * ((C - 1) - a_full)
    starts = [1]
    for c in range(1, C):
        starts.append(starts[c - 1] + own_len[c - 1])
    assert starts[-1] + own_len[-1] - 1 == T - 1
    tbase = [1] + [starts[c] - W for c in range(1, C)]
    return own_len, tbase


def _build():
    """Build + compile the per-core Bass program (identical across cores)."""
    from concourse import bacc, mybir
    import concourse.tile as tile

    nc = bacc.Bacc("TRN2", target_bir_lowering=False, debug=False)
    bf = mybir.dt.bfloat16
    f32 = mybir.dt.float32
    f8 = mybir.dt.float8e4
    DR = mybir.MatmulPerfMode.DoubleRow

    # A' = fp8(32*A) packed as 8 DoubleRow blocks b=(K*4+m): [128, 2, 128]
    # with a8[p, b, i, m'] = A'[(2K+i)*128+p, m*128+m'].
    a_d = nc.dram_tensor("a8", (128, 8, 2, 128), f8, kind="ExternalInput").ap()
    # Bem^T tiled 4x vertically (K=128 one-hot emission matmuls; one-hot rows
    # offset by 32*(iter%4) to select a replica).
    bemt_d = nc.dram_tensor("bemt8", (128, S), f8, kind="ExternalInput").ap()
    x_d = nc.dram_tensor("x8", (128, ITERS, N), f8, kind="ExternalInput").ap()
    # init8[p, kt, col] = alpha_init[kt*128+p, col]
    init_d = nc.dram_tensor("init8", (128, 4, N), f8, kind="ExternalInput").ap()
    out_d = nc.dram_tensor("snaps8", (len(SNAPS), 128, 4, N), f8,
                           kind="ExternalOutput").ap()

    with tile.TileContext(nc) as tc, ExitStack() as ctx:
        consts = ctx.enter_context(tc.tile_pool(name="consts", bufs=1))
        alphap = ctx.enter_context(tc.tile_pool(name="alpha", bufs=2))
        emp = ctx.enter_context(tc.tile_pool(name="em", bufs=4))
        pscan = ctx.enter_context(tc.tile_pool(name="pscan", bufs=1, space="PSUM"))
        pem = ctx.enter_context(tc.tile_pool(name="pem", bufs=2, space="PSUM"))

        # Scan PSUM: one tile per output m-block so each DVE multiply starts
        # as soon as its own 2 matmuls finish (Tile tracks deps at tile
        # granularity - coarser tiles serialize the recurrence).
        ps_m = [
            pscan.tile([128, N], f32, tag=f"ps{m}", name=f"ps{m}")
            for m in range(4)
        ]

        # PE warmup: HAM keeps the PE at 1.2 GHz until ~3.4us of sustained
        # ARRAY activity.  Dense full-array dummy matmuls on a zeroed tile
        # warm it while input DMAs are in flight (WAR deps against the first
        # real scan matmuls order them).
        dummy_w = consts.tile([128, S], bf, tag="dummy", name="dummy_w")
        nc.vector.memset(dummy_w, 0.0)
        dummy_n = [0]

        def emit_dummy(count):
            for _ in range(count):
                r = dummy_n[0] % 4
                dummy_n[0] += 1
                nc.tensor.matmul(
                    ps_m[r][:], dummy_w[:, 0:128], dummy_w[:],
                    start=True, stop=True,
                )

        emit_dummy(6)

        # Input loads: em dependencies (Bem, first X slices) first, then
        # init/A so the first scan iteration can start, then the X tail.
        bemt_sb = consts.tile([128, S], f8, tag="bemt", name="bemt")
        nc.default_dma_engine.dma_start(out=bemt_sb, in_=bemt_d[:, :])
        x_sb = consts.tile([128, ITERS, N], f8, tag="xoh", name="xoh")
        nc.default_dma_engine.dma_start(out=x_sb[:, 0:4, :], in_=x_d[:, 0:4, :])
        init_sb = consts.tile([128, 4, N], f8, tag="init", name="init_sb")
        nc.default_dma_engine.dma_start(out=init_sb, in_=init_d[:, :, :])
        a_sb = consts.tile([128, 8, 2, 128], f8, tag="a", name="a_sb")
        nc.default_dma_engine.dma_start(out=a_sb, in_=a_d[:, :, :, :])
        nc.default_dma_engine.dma_start(
            out=x_sb[:, 4:ITERS, :], in_=x_d[:, 4:ITERS, :]
        )

        def emit_em(i, prologue=False):
            """Emission products for iter i: 4 one-hot matmuls + 2 ACT copies.

            pem bufs=2 paces the matmuls against the copies; em_t bufs=4
            holds iters i..i+3 (emitted 3 iterations ahead in the loop)."""
            halves = []
            for h in range(2):
                ep = pem.tile([128, 2 * N], f32, tag="pem", name=f"ep_{i}_{h}")
                for j in range(2):
                    m = 2 * h + j
                    nc.tensor.matmul(
                        ep[:, j * N:(j + 1) * N],
                        bemt_sb[:, m * 128:(m + 1) * 128],
                        x_sb[:, i, :],
                        start=True, stop=True,
                    )
                et = emp.tile([128, 2 * N], bf, tag=f"emh{h}", name=f"em_{i}_{h}")
                nc.scalar.copy(et[:], ep[:])
                if prologue:
                    emit_dummy(3)
                halves.append(et)
            return halves

        em_tiles = {j: emit_em(j, prologue=True) for j in range(3)}

        # alpha views: kpair K covers state rows (2K+i)*128+p, i in {0,1}
        alpha = [init_sb[:, 0:2, :], init_sb[:, 2:4, :]]

        snap_row = 0
        for i in range(ITERS):
            # Emission products for iter i+3 first: independent of alpha,
            # they fill the PE while the DVE finishes iter i-1's multiplies.
            if i + 3 < ITERS:
                em_tiles[i + 3] = emit_em(i + 3)
            # Scan: DoubleRow fp8 (K=256 per matmul).  The first two matmuls
            # consume only alpha[0] so iter i-1's alpha[1] multiply gets an
            # extra ~2 matmul slots of slack.
            for m, K in ((0, 0), (1, 0), (0, 1), (1, 1),
                         (2, 0), (3, 0), (2, 1), (3, 1)):
                nc.tensor.matmul(
                    ps_m[m][:],
                    a_sb[:, K * 4 + m, :, :],
                    alpha[K],
                    start=(K == 0), stop=(K == 1),
                    perf_mode=DR,
                )
            new_alpha = [
                alphap.tile([128, 2, N], f8, tag=f"al{K}", name=f"al_{i}_{K}")
                for K in range(2)
            ]
            for m in range(4):
                K, j = m // 2, m % 2
                nc.vector.tensor_mul(
                    new_alpha[K][:, j:j + 1, :],
                    ps_m[m][:],
                    em_tiles[i][K][:, j * N:(j + 1) * N],
                )
            del em_tiles[i]
            alpha = new_alpha
            if i in SNAPS:
                for K in range(2):
                    nc.default_dma_engine.dma_start(
                        out=out_d[snap_row, :, K * 2:(K + 1) * 2, :],
                        in_=alpha[K],
                    )
                snap_row += 1

    nc.compile()
    return nc


def _get_nc():
    if "nc" not in _CACHE:
        _CACHE["nc"] = _build()
    return _CACHE["nc"]


def _pack(inputs, A, Bem, pi):
    """Host-side input prep: shard chunks over cores, build fp8 inputs."""
    own_len, tbase = _plan()
    obs = np.ascontiguousarray(np.argmax(inputs, axis=-1))  # [B, T]

    # A' = fp8(32*A) -> DoubleRow blocks [128, 8, 2, 128]
    Aq = (np.float32(A_SCALE) * A).astype(E4)
    A4 = Aq.reshape(4, 128, 4, 128)            # [kt, p, m, m']
    a8 = np.empty((128, 8, 2, 128), E4)
    for Kp in range(2):
        for m in range(4):
            for i2 in range(2):
                a8[:, Kp * 4 + m, i2, :] = A4[2 * Kp + i2, :, m, :]
    a8 = np.ascontiguousarray(a8)

    bemt8 = np.ascontiguousarray(np.tile(Bem.astype(E4).T, (4, 1)))  # [128, S]

    # chunk-0 init column (true normalized alpha_0), other chunks uniform.
    em0 = Bem[np.arange(S)[:, None], obs[None, :, 0]]       # [S, B]
    alpha0 = pi[:, None] * em0
    z0 = alpha0.sum(axis=0, dtype=np.float64)               # [B]
    alpha0n = alpha0 / z0.astype(np.float32)

    tb = np.asarray(tbase)
    in_maps = []
    s0_chunk0 = None
    for core in range(NCORES):
        tbs = tb[core * NCH:(core + 1) * NCH]               # [NCH]
        t_idx = np.clip(tbs[None, :] + np.arange(ITERS)[:, None], 1, T - 1)
        sym = obs[:, t_idx]                                 # [B, ITERS, NCH]
        sym = np.moveaxis(sym, 0, 2)                        # [ITERS, NCH, B]
        sym = sym.reshape(ITERS, N)
        sym = sym + (np.arange(ITERS) % 4)[:, None] * E     # replica row offset
        x8 = (sym[None, :, :] == np.arange(128)[:, None, None]).astype(E4)
        x8 = np.ascontiguousarray(x8)                       # [128, ITERS, N]

        init = np.full((S, N), INIT_SCALE / np.float32(S), np.float32)
        if core == 0:
            init[:, 0:B] = alpha0n * INIT_SCALE
        init8 = init.astype(E4)
        if core == 0:
            s0_chunk0 = np.log(init8[:, 0:B].astype(np.float64).sum(axis=0))
        init8 = np.ascontiguousarray(
            init8.reshape(4, 128, N).transpose(1, 0, 2)     # [p, kt, col]
        )
        in_maps.append({
            "a8": a8,
            "bemt8": bemt8,
            "x8": x8,
            "init8": init8,
        })

    host = {"own_len": own_len, "z0": z0, "s0_chunk0": s0_chunk0}
    return in_maps, host


def _assemble(results, host):
    """Combine per-core alpha snapshots into loglik [B] (float64 host math)."""
    own_len = host["own_len"]
    logc = np.log(np.float64(A_SCALE))
    loglik = np.log(host["z0"]).copy()                      # [B]
    for core in range(NCORES):
        snaps = results[core]["snaps8"]                     # [3, 128, 4, N] fp8
        al = snaps.astype(np.float64).transpose(0, 2, 1, 3).reshape(3, S, N)
        lz = np.log(al.sum(axis=1))                         # [3, N]
        for cl in range(NCH):
            c = core * NCH + cl
            cols = slice(cl * B, (cl + 1) * B)
            if c == 0:
                nst = own_len[0]
                loglik += lz[2, cols] - host["s0_chunk0"] - nst * logc
            else:
                row = 2 if own_len[c] == L else 1
                nst = (ITERS - 1 if row == 2 else ITERS - 2) - (W - 1)
                loglik += lz[row, cols] - lz[0, cols] - nst * logc
    return loglik.astype(np.float32)


def run(inputs, A, Bem, pi, trace=False):
    from concourse import bass_utils

    nc = _get_nc()
    in_maps, host = _pack(
        np.asarray(inputs, np.float32), np.asarray(A, np.float32),
        np.asarray(Bem, np.float32), np.asarray(pi, np.float32),
    )
    res = bass_utils.run_bass_kernel_spmd(
        nc, in_maps, core_ids=list(range(NCORES)), trace=trace
    )
    loglik = _assemble(res.results, host)
    return loglik, res


def kernel(inputs, A, Bem, pi):
    loglik, _ = run(inputs, A, Bem, pi, trace=False)
    return loglik
